# revision 28
# baseline (speedup 1.0000x reference)
"""MoE grouped-GEMM (8 experts) on 8 Trainium2 NeuronCores.

Problem: input [32768, 1024] routed contiguously to 8 experts (counts in
num_experts_per_token); expert i computes x_i @ W_i.T + b_i with
W [8, 4096, 1024], b [8, 4096]. Output [32768, 4096].

Sharding: expert-parallel, expert i <-> core i, zero collectives. Host
slices each expert's token block and packs operands into exact SBUF tile
layouts; each core runs a 4096x1024x4096 GEMM; host adds bias (fp32) and
concatenates.

Per-core kernel, PE-bound. Mixed-precision contraction split (rel err
1.895e-2 measured end-to-end vs the 2e-2 gate):
  - k-tiles 2..7 (K=768): x bf16 STATIONARY x w bf16 MOVING, plain
    matmuls [K128, M128, N512] at 216 ns (512 cyc @ 2.4 GHz + NX).
    k-outer/n-inner reuses each stationary tile for 8 consecutive
    matmuls so LDWEIGHTS (97 ns w/ FWL) hides completely.
  - k-tiles 0..1 (K=256): ONE fp8 e4m3 DoubleRow matmul (lhsT
    [128,2,128], rhs [128,2,512]) replaces two plain matmuls —
    ~0.5x the cycles for that quarter of the contraction.
  - Both parts accumulate into the SAME PSUM bank: all operands are
    pre-scaled per-expert so every partial product carries the SAME
    fixed factor a*b = 2^16 (a,b balanced so x*a and w*b both peak
    ~189 < 240 = e4m3 max). The drain is then a single scalar
    multiply by 2^-16 (constant, shared across cores) fused with the
    PSUM->SBUF bf16 copy, alternating DVE / ACT two-wide.
  - Weights resident in SBUF (6 MB bf16 + 1 MB fp8), single pass over
    x; supply is deadline-spread across the sync HWDGE ring
    (w0/w2/w4/w8), the scalar HWDGE ring (w1/w3 between x slices) and
    the slow gpsimd SWDGE ring (w5 only); first k-tile sliced so the
    first matmuls gate on 64 KB. x m-blocks stream 2-ahead on the
    scalar ring. The first block is chip-HBM-bound regardless (all 8
    cores pull 7 MB at once) — ~8 us one-time fill stall.
  - y staged bf16, paired into [128, 1024] tiles (2 KB/partition DMA
    lines), alternating scalar/sync rings; host upcasts to fp32 and
    adds bias. Last m-block runs n-outer/k-inner so drains + output
    DMA overlap the tail matmuls.
"""

import sys

if "/opt/trn_rl_repo" not in sys.path:
    sys.path.insert(0, "/opt/trn_rl_repo")

import numpy as np

E, T, DIN, DOUT = 8, 32768, 1024, 4096
NCORES = 8
TOKC = T // NCORES  # tokens per core (capacity)

KT = 128   # contraction tile (SBUF partitions)
MT = 128   # token tile (PSUM partitions)
NT = 512   # dout tile (one fp32 PSUM bank)
KTILES = DIN // KT    # 8
MTILES = TOKC // MT   # 32
NTILES = DOUT // NT   # 8

KF8 = 256           # contraction cols computed in fp8 DoubleRow (k-tiles 0,1)
K16T = (DIN - KF8) // KT  # 6 bf16 k-tiles (2..7)
PSCALE = 2.0 ** 16  # fixed a*b product; drain multiplies by 1/PSCALE
F8MAX = 240.0       # float8_e4m3 max normal
F8MARGIN = 0.98

_CACHE = {}


def _build_nc():
    import concourse.bacc as bacc
    import concourse.tile as tile
    import concourse.mybir as mybir

    nc = bacc.Bacc("TRN2", target_bir_lowering=False, debug=False,
                   num_devices=NCORES)

    # x16B[m][kk, j*MT + t] = bf16(x[m*MT + t, (j+2)*KT + kk] * a)
    x16B = nc.dram_tensor("x16B", [MTILES, KT, K16T * MT], mybir.dt.bfloat16,
                          kind="ExternalInput")
    # x8B[m][kk, i*MT + t] = e4m3(x[m*MT + t, i*KT + kk] * a), i in {0,1}
    x8B = nc.dram_tensor("x8B", [MTILES, KT, 2 * MT], mybir.dt.float8e4,
                         kind="ExternalInput")
    # w16B[j][kk, d] = bf16(w[d, (j+2)*KT + kk] * b)
    w16B = nc.dram_tensor("w16B", [K16T, KT, DOUT], mybir.dt.bfloat16,
                          kind="ExternalInput")
    # w8B[kk, i*DOUT + d] = e4m3(w[d, i*KT + kk] * b)
    w8B = nc.dram_tensor("w8B", [KT, 2 * DOUT], mybir.dt.float8e4,
                         kind="ExternalInput")
    y = nc.dram_tensor("y", [TOKC, DOUT], mybir.dt.bfloat16,
                       kind="ExternalOutput")

    DR = mybir.MatmulPerfMode.DoubleRow
    INV = 1.0 / PSCALE

    with tile.TileContext(nc) as tc:
        with (
            tc.tile_pool(name="wpool", bufs=1) as wpool,
            tc.tile_pool(name="xpool", bufs=4) as xpool,
            tc.tile_pool(name="opool", bufs=12) as opool,
            tc.tile_pool(name="psum", bufs=8, space="PSUM") as psum_pool,
        ):
            # resident weights. Even-k bf16 tiles on the sync HWDGE
            # ring, odd-k + the fp8 pair tile on the gpsimd SWDGE ring
            # (parallel triggers); k=2 sliced so the first matmuls gate
            # on 64 KB. ~600 ns/trigger makes big DMAs essential.
            wt = [wpool.tile([KT, DOUT], mybir.dt.bfloat16,
                             name=f"wt{j}", tag=f"wt{j}")
                  for j in range(K16T)]
            w8t = wpool.tile([KT, 2, DOUT], mybir.dt.float8e4,
                             name="w8t", tag="w8t")
            # w supply is deadline-driven for m-block 0 (7 MB wanted in
            # ~10 us): sync ring w0/w2/w4/w8t, scalar ring interleaves
            # w1/w3 between the x slices below, slow gpsimd SWDGE
            # (~90 GB/s) gets only the latest-needed tile w5.
            nc.sync.dma_start(wt[0][:, 0:NT], w16B[0][:, 0:NT])
            nc.sync.dma_start(wt[0][:, NT:], w16B[0][:, NT:])
            nc.sync.dma_start(wt[2][:], w16B[2])
            nc.sync.dma_start(wt[4][:], w16B[4])
            nc.sync.dma_start(w8t[:], w8B[:])
            nc.gpsimd.dma_start(wt[5][:], w16B[5])

            def load_x16(m, sliced=False):
                # scalar (ACT) HWDGE ring, parallel to weights
                t16 = xpool.tile([KT, K16T, MT], mybir.dt.bfloat16,
                                 name="xm16", tag="xm16")
                if sliced:  # k-pair slices (512 B/partition) to gate fast
                    for j2 in range(0, K16T, 2):
                        nc.scalar.dma_start(t16[:, j2:j2 + 2, :],
                                            x16B[m][:, j2 * MT:(j2 + 2) * MT])
                else:
                    nc.scalar.dma_start(t16[:], x16B[m])
                return t16

            def load_x8(m):
                t8 = xpool.tile([KT, 2, MT], mybir.dt.float8e4,
                                name="xm8", tag="xm8")
                nc.scalar.dma_start(t8[:], x8B[m])
                return t8

            def load_xm(m):
                return load_x16(m), load_x8(m)

            # m0's x16 first (gates mm#0), then w1/w3 interleaved ahead
            # of the later-needed x8/m1 loads
            x16_0 = load_x16(0, sliced=True)
            nc.scalar.dma_start(wt[1][:], w16B[1])
            x8_0 = load_x8(0)
            nc.scalar.dma_start(wt[3][:], w16B[3])
            xm_cur = (x16_0, x8_0)
            xm_next = load_xm(1)

            def drain(m, n, accs, ot):
                # fused descale (x 2^-16) + PSUM->SBUF bf16 copy (bias
                # is added on the host), alternating DVE / ACT so
                # drains pipeline two-wide; after the odd half, one
                # 256 KB (2 KB/partition) DMA, alternating between the
                # scalar and sync rings.
                half = (n % 2) * NT
                if n % 2 == 0:
                    nc.vector.tensor_scalar_mul(
                        ot[:, half:half + NT], accs[n][:], INV)
                else:
                    nc.scalar.activation(
                        ot[:, half:half + NT], accs[n][:],
                        mybir.ActivationFunctionType.Identity, scale=INV)
                if n % 2:
                    u = n // 2
                    eng = nc.scalar if (m * 4 + u) % 2 == 0 else nc.sync
                    row0 = m * MT
                    eng.dma_start(
                        y[row0:row0 + MT, (n - 1) * NT:(n + 1) * NT], ot[:])

            def mm16(accs, x16t, j, n, start, stop):
                nc.tensor.matmul(
                    accs[n][:], x16t[:, j, :], wt[j][:, n * NT:(n + 1) * NT],
                    start=start, stop=stop)

            def mm8(accs, x8t, n, start, stop):
                nc.tensor.matmul(
                    accs[n][:], x8t[:], w8t[:, :, n * NT:(n + 1) * NT],
                    start=start, stop=stop, perf_mode=DR)

            for m in range(MTILES):
                if m + 2 < MTILES:
                    xm_fut = load_xm(m + 2)
                else:
                    xm_fut = None
                x16t, x8t = xm_cur
                accs = [psum_pool.tile([MT, NT], mybir.dt.float32,
                                       name="acc", tag="acc")
                        for n in range(NTILES)]
                last_m = m == MTILES - 1
                if not last_m:
                    # k-outer/n-inner: stationary tile reused by 8
                    # consecutive matmuls; all 8 PSUM banks accumulate.
                    # The fp8 DoubleRow pass alternates ends by block
                    # parity (even: last, odd: first) so adjacent
                    # blocks' DR sections run back-to-back — one
                    # DR<->bf16 LDW-exposing transition per block
                    # instead of two.
                    if m % 2 == 0:
                        for j in range(K16T):
                            for n in range(NTILES):
                                mm16(accs, x16t, j, n, j == 0, False)
                        for n in range(NTILES):
                            mm8(accs, x8t, n, False, True)
                    else:
                        for n in range(NTILES):
                            mm8(accs, x8t, n, True, False)
                        for j in range(K16T):
                            for n in range(NTILES):
                                mm16(accs, x16t, j, n, False,
                                     j == K16T - 1)
                    for n in range(NTILES):
                        if n % 2 == 0:
                            ot = opool.tile([MT, 2 * NT], mybir.dt.bfloat16,
                                            name="ot", tag="ot")
                        drain(m, n, accs, ot)
                else:
                    # last block n-outer/k-inner so drains + output DMA
                    # overlap the remaining matmuls (short tail)
                    for n in range(NTILES):
                        mm8(accs, x8t, n, True, False)
                        for j in range(K16T):
                            mm16(accs, x16t, j, n, False, j == K16T - 1)
                        if n % 2 == 0:
                            ot = opool.tile([MT, 2 * NT], mybir.dt.bfloat16,
                                            name="ot", tag="ot")
                        drain(m, n, accs, ot)
                xm_cur, xm_next = xm_next, xm_fut

    nc.compile()
    return nc


def _install_neff_cache():
    """Disk-cache walrus NEFF compiles keyed on the BIR bytes."""
    if _CACHE.get("neff_cache_installed"):
        return
    _CACHE["neff_cache_installed"] = True
    import hashlib
    import os
    import shutil

    import concourse.bass2jax as bass2jax

    cache_dir = "/root/.neff_bir_cache"
    os.makedirs(cache_dir, exist_ok=True)
    orig = bass2jax.compile_bir_kernel

    def cached_compile(ant_bir_str, tmpdir, neff_name="file.neff", **kw):
        key = hashlib.sha256(
            ant_bir_str if isinstance(ant_bir_str, bytes)
            else ant_bir_str.encode()).hexdigest()
        hit = os.path.join(cache_dir, key + ".neff")
        dst = os.path.join(tmpdir, neff_name)
        if os.path.exists(hit):
            shutil.copyfile(hit, dst)
            return dst
        out = orig(ant_bir_str, tmpdir, neff_name=neff_name, **kw)
        try:
            shutil.copyfile(out, hit)
        except OSError:
            pass
        return out

    bass2jax.compile_bir_kernel = cached_compile


def _get_nc():
    if "nc" not in _CACHE:
        _install_neff_cache()
        _CACHE["nc"] = _build_nc()
    return _CACHE["nc"]


def kernel(input, weight, bias, num_experts_per_token):
    import ml_dtypes
    from concourse.bass_utils import run_bass_kernel_spmd

    input = np.ascontiguousarray(np.asarray(input, dtype=np.float32))
    weight = np.ascontiguousarray(np.asarray(weight, dtype=np.float32))
    bias = np.ascontiguousarray(np.asarray(bias, dtype=np.float32))
    counts = np.asarray(num_experts_per_token).astype(np.int64)
    offsets = np.concatenate([[0], np.cumsum(counts)]).astype(np.int64)

    if counts.max() > TOKC:
        # capacity overflow (never hit with balanced routing): numpy fallback
        outs = []
        for i in range(E):
            xi = input[offsets[i]:offsets[i + 1]]
            outs.append(xi @ weight[i].T + bias[i])
        return np.concatenate(outs, axis=0)

    bf16 = ml_dtypes.bfloat16
    e4m3 = ml_dtypes.float8_e4m3

    in_maps = []
    for i in range(E):
        wi = weight[i]                                  # [DOUT, DIN]
        xi = input[offsets[i]:offsets[i + 1]]           # [n_i, DIN]
        if xi.shape[0] < TOKC:
            xi = np.concatenate(
                [xi, np.zeros((TOKC - xi.shape[0], DIN), np.float32)], axis=0)

        # balanced scales with fixed product a*b = PSCALE so the drain
        # constant is shared across cores; clamp a into the range that
        # keeps BOTH fp8 operands in [0, F8MAX].
        mx = float(np.abs(xi[:, :KF8]).max())
        mw = float(np.abs(wi[:, :KF8]).max())
        if mx > 0 and mw > 0:
            a = float(np.sqrt(PSCALE * mw / mx))
            a = min(max(a, PSCALE * mw / (F8MAX * F8MARGIN)),
                    F8MAX * F8MARGIN / mx)
        else:
            a = 1.0
        b = PSCALE / a

        xs = xi * a
        # [m, kk, j, t] <- xs[m*128+t, (j+2)*128+kk]
        x16p = np.ascontiguousarray(
            xs[:, KF8:].reshape(MTILES, MT, K16T, KT)
            .transpose(0, 3, 2, 1)
            .reshape(MTILES, KT, K16T * MT)
            .astype(bf16))
        # [m, kk, i, t] <- xs[m*128+t, i*128+kk]
        x8p = np.ascontiguousarray(
            xs[:, :KF8].reshape(MTILES, MT, 2, KT)
            .transpose(0, 3, 2, 1)
            .reshape(MTILES, KT, 2 * MT)
            .astype(e4m3))
        ws = (wi * b).T                                 # [DIN, DOUT]
        # [j, kk, d] <- ws[(j+2)*128+kk, d]
        w16p = np.ascontiguousarray(
            ws[KF8:].reshape(K16T, KT, DOUT).astype(bf16))
        # [kk, i, d] <- ws[i*128+kk, d]
        w8p = np.ascontiguousarray(
            ws[:KF8].reshape(2, KT, DOUT)
            .transpose(1, 0, 2)
            .reshape(KT, 2 * DOUT)
            .astype(e4m3))
        in_maps.append({"x16B": x16p, "x8B": x8p,
                        "w16B": w16p, "w8B": w8p})

    nc = _get_nc()
    import os
    trace = bool(int(os.environ.get("KERNEL_TRACE", "0")))
    if trace:
        try:
            import axon_profile_shim
            axon_profile_shim.install()
            import antenv.axon_hooks  # noqa: F401
        except Exception:
            trace = False
    res = run_bass_kernel_spmd(nc, in_maps, core_ids=list(range(NCORES)),
                               trace=trace)
    _CACHE["last_result"] = res

    out = np.empty((T, DOUT), dtype=np.float32)
    pos = 0
    for i in range(E):
        n_i = int(counts[i])
        # bias is added here (host, fp32) rather than on-device
        out[pos:pos + n_i] = res.results[i]["y"][:n_i].astype(np.float32)
        out[pos:pos + n_i] += bias[i]
        pos += n_i
    return out


# revision 30
# speedup vs baseline: 1.0062x; 1.0062x over previous
"""MoE grouped-GEMM (8 experts) on 8 Trainium2 NeuronCores.

Problem: input [32768, 1024] routed contiguously to 8 experts (counts in
num_experts_per_token); expert i computes x_i @ W_i.T + b_i with
W [8, 4096, 1024], b [8, 4096]. Output [32768, 4096].

Sharding: expert-parallel, expert i <-> core i, zero collectives. Host
slices each expert's token block and packs operands into exact SBUF tile
layouts; each core runs a 4096x1024x4096 GEMM; host adds bias (fp32) and
concatenates.

Per-core kernel, PE-bound. Mixed-precision contraction split (rel err
1.895e-2 measured end-to-end vs the 2e-2 gate):
  - k-tiles 2..7 (K=768): x bf16 STATIONARY x w bf16 MOVING, plain
    matmuls [K128, M128, N512] at 216 ns (512 cyc @ 2.4 GHz + NX).
    k-outer/n-inner reuses each stationary tile for 8 consecutive
    matmuls so LDWEIGHTS (97 ns w/ FWL) hides completely.
  - k-tiles 0..1 (K=256): ONE fp8 e4m3 DoubleRow matmul (lhsT
    [128,2,128], rhs [128,2,512]) replaces two plain matmuls —
    ~0.5x the cycles for that quarter of the contraction.
  - Both parts accumulate into the SAME PSUM bank: all operands are
    pre-scaled per-expert so every partial product carries the SAME
    fixed factor a*b = 2^16 (a,b balanced so x*a and w*b both peak
    ~189 < 240 = e4m3 max). The drain is then a single scalar
    multiply by 2^-16 (constant, shared across cores) fused with the
    PSUM->SBUF bf16 copy, alternating DVE / ACT two-wide.
  - Weights resident in SBUF (6 MB bf16 + 1 MB fp8), single pass over
    x; supply is deadline-spread across the sync HWDGE ring
    (w0/w2/w4/w8), the scalar HWDGE ring (w1/w3 between x slices) and
    the slow gpsimd SWDGE ring (w5 only); first k-tile sliced so the
    first matmuls gate on 64 KB. x m-blocks stream 2-ahead on the
    scalar ring. The first block is chip-HBM-bound regardless (all 8
    cores pull 7 MB at once) — ~8 us one-time fill stall.
  - y staged bf16, paired into [128, 1024] tiles (2 KB/partition DMA
    lines), alternating scalar/sync rings; host upcasts to fp32 and
    adds bias. Last m-block runs n-outer/k-inner so drains + output
    DMA overlap the tail matmuls.
"""

import sys

if "/opt/trn_rl_repo" not in sys.path:
    sys.path.insert(0, "/opt/trn_rl_repo")

import numpy as np

E, T, DIN, DOUT = 8, 32768, 1024, 4096
NCORES = 8
TOKC = T // NCORES  # tokens per core (capacity)

KT = 128   # contraction tile (SBUF partitions)
MT = 128   # token tile (PSUM partitions)
NT = 512   # dout tile (one fp32 PSUM bank)
KTILES = DIN // KT    # 8
MTILES = TOKC // MT   # 32
NTILES = DOUT // NT   # 8

KF8 = 256           # contraction cols computed in fp8 DoubleRow (k-tiles 0,1)
K16T = (DIN - KF8) // KT  # 6 bf16 k-tiles (2..7)
PSCALE = 2.0 ** 16  # fixed a*b product; drain multiplies by 1/PSCALE
F8MAX = 240.0       # float8_e4m3 max normal
F8MARGIN = 0.98

_CACHE = {}


def _build_nc():
    import concourse.bacc as bacc
    import concourse.tile as tile
    import concourse.mybir as mybir

    nc = bacc.Bacc("TRN2", target_bir_lowering=False, debug=False,
                   num_devices=NCORES)

    # x16B[m][kk, j*MT + t] = bf16(x[m*MT + t, (j+2)*KT + kk] * a)
    x16B = nc.dram_tensor("x16B", [MTILES, KT, K16T * MT], mybir.dt.bfloat16,
                          kind="ExternalInput")
    # x8B[m][kk, i*MT + t] = e4m3(x[m*MT + t, i*KT + kk] * a), i in {0,1}
    x8B = nc.dram_tensor("x8B", [MTILES, KT, 2 * MT], mybir.dt.float8e4,
                         kind="ExternalInput")
    # w16B[j][kk, d] = bf16(w[d, (j+2)*KT + kk] * b)
    w16B = nc.dram_tensor("w16B", [K16T, KT, DOUT], mybir.dt.bfloat16,
                          kind="ExternalInput")
    # w8B[kk, i*DOUT + d] = e4m3(w[d, i*KT + kk] * b)
    w8B = nc.dram_tensor("w8B", [KT, 2 * DOUT], mybir.dt.float8e4,
                         kind="ExternalInput")
    y = nc.dram_tensor("y", [TOKC, DOUT], mybir.dt.bfloat16,
                       kind="ExternalOutput")

    DR = mybir.MatmulPerfMode.DoubleRow
    INV = 1.0 / PSCALE

    with tile.TileContext(nc) as tc:
        with (
            tc.tile_pool(name="wpool", bufs=1) as wpool,
            tc.tile_pool(name="xpool", bufs=4) as xpool,
            tc.tile_pool(name="opool", bufs=12) as opool,
            tc.tile_pool(name="psum", bufs=8, space="PSUM") as psum_pool,
        ):
            # resident weights. Even-k bf16 tiles on the sync HWDGE
            # ring, odd-k + the fp8 pair tile on the gpsimd SWDGE ring
            # (parallel triggers); k=2 sliced so the first matmuls gate
            # on 64 KB. ~600 ns/trigger makes big DMAs essential.
            wt = [wpool.tile([KT, DOUT], mybir.dt.bfloat16,
                             name=f"wt{j}", tag=f"wt{j}")
                  for j in range(K16T)]
            w8t = wpool.tile([KT, 2, DOUT], mybir.dt.float8e4,
                             name="w8t", tag="w8t")
            # w supply is deadline-driven for m-block 0 (7 MB wanted in
            # ~10 us): sync ring w0/w2/w4/w8t, scalar ring interleaves
            # w1/w3 between the x slices below, slow gpsimd SWDGE
            # (~90 GB/s) gets only the latest-needed tile w5.
            # n-half slices: matmul (j, n) consumes only a 512-col slice,
            # so half-tile DMAs unblock block 0's j-loop ~2x sooner
            H = DOUT // 2
            nc.sync.dma_start(wt[0][:, 0:NT], w16B[0][:, 0:NT])
            nc.sync.dma_start(wt[0][:, NT:H], w16B[0][:, NT:H])
            nc.sync.dma_start(wt[0][:, H:], w16B[0][:, H:])
            for j in (2, 4):
                nc.sync.dma_start(wt[j][:, 0:H], w16B[j][:, 0:H])
                nc.sync.dma_start(wt[j][:, H:], w16B[j][:, H:])
            # fp8 pair tile quartered the same way (both i-slots per half)
            for i in range(2):
                nc.sync.dma_start(w8t[:, i, 0:H], w8B[:, i * DOUT:i * DOUT + H])
            for i in range(2):
                nc.sync.dma_start(w8t[:, i, H:], w8B[:, i * DOUT + H:(i + 1) * DOUT])
            nc.gpsimd.dma_start(wt[5][:, 0:H], w16B[5][:, 0:H])
            nc.gpsimd.dma_start(wt[5][:, H:], w16B[5][:, H:])

            def load_x16(m, sliced=False):
                # scalar (ACT) HWDGE ring, parallel to weights
                t16 = xpool.tile([KT, K16T, MT], mybir.dt.bfloat16,
                                 name="xm16", tag="xm16")
                if sliced:  # k-pair slices (512 B/partition) to gate fast
                    for j2 in range(0, K16T, 2):
                        nc.scalar.dma_start(t16[:, j2:j2 + 2, :],
                                            x16B[m][:, j2 * MT:(j2 + 2) * MT])
                else:
                    nc.scalar.dma_start(t16[:], x16B[m])
                return t16

            def load_x8(m):
                t8 = xpool.tile([KT, 2, MT], mybir.dt.float8e4,
                                name="xm8", tag="xm8")
                nc.scalar.dma_start(t8[:], x8B[m])
                return t8

            def load_xm(m):
                return load_x16(m), load_x8(m)

            # m0's x16 first (gates mm#0), then w1/w3 interleaved ahead
            # of the later-needed x8/m1 loads
            x16_0 = load_x16(0, sliced=True)
            nc.scalar.dma_start(wt[1][:, 0:H], w16B[1][:, 0:H])
            nc.scalar.dma_start(wt[1][:, H:], w16B[1][:, H:])
            x8_0 = load_x8(0)
            nc.scalar.dma_start(wt[3][:, 0:H], w16B[3][:, 0:H])
            nc.scalar.dma_start(wt[3][:, H:], w16B[3][:, H:])
            xm_cur = (x16_0, x8_0)
            xm_next = load_xm(1)

            def drain(m, n, accs, ot):
                # fused descale (x 2^-16) + PSUM->SBUF bf16 copy (bias
                # is added on the host), alternating DVE / ACT so
                # drains pipeline two-wide; after the odd half, one
                # 256 KB (2 KB/partition) DMA, alternating between the
                # scalar and sync rings.
                half = (n % 2) * NT
                if n % 2 == 0:
                    nc.vector.tensor_scalar_mul(
                        ot[:, half:half + NT], accs[n][:], INV)
                else:
                    nc.scalar.activation(
                        ot[:, half:half + NT], accs[n][:],
                        mybir.ActivationFunctionType.Identity, scale=INV)
                if n % 2:
                    u = n // 2
                    eng = nc.scalar if (m * 4 + u) % 2 == 0 else nc.sync
                    row0 = m * MT
                    eng.dma_start(
                        y[row0:row0 + MT, (n - 1) * NT:(n + 1) * NT], ot[:])

            def mm16(accs, x16t, j, n, start, stop):
                nc.tensor.matmul(
                    accs[n][:], x16t[:, j, :], wt[j][:, n * NT:(n + 1) * NT],
                    start=start, stop=stop)

            def mm8(accs, x8t, n, start, stop):
                nc.tensor.matmul(
                    accs[n][:], x8t[:], w8t[:, :, n * NT:(n + 1) * NT],
                    start=start, stop=stop, perf_mode=DR)

            for m in range(MTILES):
                if m + 2 < MTILES:
                    xm_fut = load_xm(m + 2)
                else:
                    xm_fut = None
                x16t, x8t = xm_cur
                accs = [psum_pool.tile([MT, NT], mybir.dt.float32,
                                       name="acc", tag="acc")
                        for n in range(NTILES)]
                last_m = m == MTILES - 1
                if not last_m:
                    # k-outer/n-inner: stationary tile reused by 8
                    # consecutive matmuls; all 8 PSUM banks accumulate.
                    # The fp8 DoubleRow pass alternates ends by block
                    # parity (even: last, odd: first) so adjacent
                    # blocks' DR sections run back-to-back — one
                    # DR<->bf16 LDW-exposing transition per block
                    # instead of two.
                    if m % 2 == 0:
                        for j in range(K16T):
                            for n in range(NTILES):
                                mm16(accs, x16t, j, n, j == 0, False)
                        for n in range(NTILES):
                            mm8(accs, x8t, n, False, True)
                    else:
                        for n in range(NTILES):
                            mm8(accs, x8t, n, True, False)
                        for j in range(K16T):
                            for n in range(NTILES):
                                mm16(accs, x16t, j, n, False,
                                     j == K16T - 1)
                    for n in range(NTILES):
                        if n % 2 == 0:
                            ot = opool.tile([MT, 2 * NT], mybir.dt.bfloat16,
                                            name="ot", tag="ot")
                        drain(m, n, accs, ot)
                else:
                    # last block n-outer/k-inner so drains + output DMA
                    # overlap the remaining matmuls (short tail)
                    for n in range(NTILES):
                        mm8(accs, x8t, n, True, False)
                        for j in range(K16T):
                            mm16(accs, x16t, j, n, False, j == K16T - 1)
                        if n % 2 == 0:
                            ot = opool.tile([MT, 2 * NT], mybir.dt.bfloat16,
                                            name="ot", tag="ot")
                        drain(m, n, accs, ot)
                xm_cur, xm_next = xm_next, xm_fut

    nc.compile()
    return nc


def _install_neff_cache():
    """Disk-cache walrus NEFF compiles keyed on the BIR bytes."""
    if _CACHE.get("neff_cache_installed"):
        return
    _CACHE["neff_cache_installed"] = True
    import hashlib
    import os
    import shutil

    import concourse.bass2jax as bass2jax

    cache_dir = "/root/.neff_bir_cache"
    os.makedirs(cache_dir, exist_ok=True)
    orig = bass2jax.compile_bir_kernel

    def cached_compile(ant_bir_str, tmpdir, neff_name="file.neff", **kw):
        key = hashlib.sha256(
            ant_bir_str if isinstance(ant_bir_str, bytes)
            else ant_bir_str.encode()).hexdigest()
        hit = os.path.join(cache_dir, key + ".neff")
        dst = os.path.join(tmpdir, neff_name)
        if os.path.exists(hit):
            shutil.copyfile(hit, dst)
            return dst
        out = orig(ant_bir_str, tmpdir, neff_name=neff_name, **kw)
        try:
            shutil.copyfile(out, hit)
        except OSError:
            pass
        return out

    bass2jax.compile_bir_kernel = cached_compile


def _get_nc():
    if "nc" not in _CACHE:
        _install_neff_cache()
        _CACHE["nc"] = _build_nc()
    return _CACHE["nc"]


def kernel(input, weight, bias, num_experts_per_token):
    import ml_dtypes
    from concourse.bass_utils import run_bass_kernel_spmd

    input = np.ascontiguousarray(np.asarray(input, dtype=np.float32))
    weight = np.ascontiguousarray(np.asarray(weight, dtype=np.float32))
    bias = np.ascontiguousarray(np.asarray(bias, dtype=np.float32))
    counts = np.asarray(num_experts_per_token).astype(np.int64)
    offsets = np.concatenate([[0], np.cumsum(counts)]).astype(np.int64)

    if counts.max() > TOKC:
        # capacity overflow (never hit with balanced routing): numpy fallback
        outs = []
        for i in range(E):
            xi = input[offsets[i]:offsets[i + 1]]
            outs.append(xi @ weight[i].T + bias[i])
        return np.concatenate(outs, axis=0)

    bf16 = ml_dtypes.bfloat16
    e4m3 = ml_dtypes.float8_e4m3

    in_maps = []
    for i in range(E):
        wi = weight[i]                                  # [DOUT, DIN]
        xi = input[offsets[i]:offsets[i + 1]]           # [n_i, DIN]
        if xi.shape[0] < TOKC:
            xi = np.concatenate(
                [xi, np.zeros((TOKC - xi.shape[0], DIN), np.float32)], axis=0)

        # balanced scales with fixed product a*b = PSCALE so the drain
        # constant is shared across cores; clamp a into the range that
        # keeps BOTH fp8 operands in [0, F8MAX].
        mx = float(np.abs(xi[:, :KF8]).max())
        mw = float(np.abs(wi[:, :KF8]).max())
        if mx > 0 and mw > 0:
            a = float(np.sqrt(PSCALE * mw / mx))
            a = min(max(a, PSCALE * mw / (F8MAX * F8MARGIN)),
                    F8MAX * F8MARGIN / mx)
        else:
            a = 1.0
        b = PSCALE / a

        xs = xi * a
        # [m, kk, j, t] <- xs[m*128+t, (j+2)*128+kk]
        x16p = np.ascontiguousarray(
            xs[:, KF8:].reshape(MTILES, MT, K16T, KT)
            .transpose(0, 3, 2, 1)
            .reshape(MTILES, KT, K16T * MT)
            .astype(bf16))
        # [m, kk, i, t] <- xs[m*128+t, i*128+kk]
        x8p = np.ascontiguousarray(
            xs[:, :KF8].reshape(MTILES, MT, 2, KT)
            .transpose(0, 3, 2, 1)
            .reshape(MTILES, KT, 2 * MT)
            .astype(e4m3))
        ws = (wi * b).T                                 # [DIN, DOUT]
        # [j, kk, d] <- ws[(j+2)*128+kk, d]
        w16p = np.ascontiguousarray(
            ws[KF8:].reshape(K16T, KT, DOUT).astype(bf16))
        # [kk, i, d] <- ws[i*128+kk, d]
        w8p = np.ascontiguousarray(
            ws[:KF8].reshape(2, KT, DOUT)
            .transpose(1, 0, 2)
            .reshape(KT, 2 * DOUT)
            .astype(e4m3))
        in_maps.append({"x16B": x16p, "x8B": x8p,
                        "w16B": w16p, "w8B": w8p})

    nc = _get_nc()
    import os
    trace = bool(int(os.environ.get("KERNEL_TRACE", "0")))
    if trace:
        try:
            import axon_profile_shim
            axon_profile_shim.install()
            import antenv.axon_hooks  # noqa: F401
        except Exception:
            trace = False
    res = run_bass_kernel_spmd(nc, in_maps, core_ids=list(range(NCORES)),
                               trace=trace)
    _CACHE["last_result"] = res

    out = np.empty((T, DOUT), dtype=np.float32)
    pos = 0
    for i in range(E):
        n_i = int(counts[i])
        # bias is added here (host, fp32) rather than on-device
        out[pos:pos + n_i] = res.results[i]["y"][:n_i].astype(np.float32)
        out[pos:pos + n_i] += bias[i]
        pos += n_i
    return out
